# revision 1
# baseline (speedup 1.0000x reference)
"""Trainium2 Bass kernel: scatter rho[b, i, j] -> out[b, fock_idx[i], fock_idx[j]].

Sharding: batch dim B across the 8 NeuronCores (pure data parallel). fock_idx is
known on the host at call time, so the scatter addressing is baked into the
compiled program as static DMA/compute access patterns.

Per-core algorithm (out is [D, D], zero except out[idx[i], idx[j]] = rho[i, j]):
  - The runtime hands the NEFF a zero-initialized ExternalOutput buffer (both
    the native run_neff path and the axon/bass2jax donation path guarantee
    this), so only rows/columns that receive data are written.
  - fock_idx (for the real problem) is strictly increasing and decomposes into
    runs of consecutive indices (32 runs of 32). Columns: each rho row is
    expanded into a [span]-wide row in SBUF with the runs placed at their
    target offsets and zeros in the gaps. Rows: each 128-row tile of rho is
    stored with one DMA per row-run to the matching block of out rows,
    touching only columns [c0, c1).
"""

import numpy as np

import concourse.bacc as bacc
import concourse.mybir as mybir
from concourse import tile
from concourse.bass_utils import run_bass_kernel_spmd

N_CORES = 8
P = 128  # SBUF partitions


def _runs(dst, src):
    """Maximal runs where dst and src both advance by 1. Yields (d0, s0, len)."""
    out = []
    d0, s0, L = int(dst[0]), int(src[0]), 1
    for k in range(1, len(dst)):
        if int(dst[k]) == d0 + L and int(src[k]) == s0 + L:
            L += 1
        else:
            out.append((d0, s0, L))
            d0, s0, L = int(dst[k]), int(src[k]), 1
    out.append((d0, s0, L))
    return out


def _build(idx, D, n):
    """Build the per-core Bass program with idx baked in."""
    f32 = mybir.dt.float32

    # Column placement: process columns in sorted-index order so the SBUF row
    # image is written left to right; a run needs source columns contiguous too.
    order = np.argsort(idx, kind="stable")
    col_runs = _runs(idx[order], order)  # (dst_col, src_col, len)
    c0 = min(r[0] for r in col_runs)
    c1 = max(r[0] + r[2] for r in col_runs)
    span = c1 - c0

    nc = bacc.Bacc("TRN2", target_bir_lowering=False, debug=False,
                   num_devices=N_CORES)
    rho = nc.dram_tensor("rho", [n, n], f32, kind="ExternalInput")
    out = nc.dram_tensor("out", [D, D], f32, kind="ExternalOutput")

    n_tiles = (n + P - 1) // P
    with tile.TileContext(nc) as tc:
        with (
            tc.tile_pool(name="rp", bufs=3) as rp,
            tc.tile_pool(name="wp", bufs=3) as wp,
        ):
            for t in range(n_tiles):
                r0 = t * P
                rows = min(P, n - r0)
                R = rp.tile([P, n], f32)
                nc.scalar.dma_start(R[:rows, :], rho[r0:r0 + rows, :])

                W = wp.tile([P, span], f32)
                nc.gpsimd.memset(W[:], 0.0)
                for d0, s0, L in col_runs:
                    nc.vector.tensor_copy(W[:rows, d0 - c0:d0 - c0 + L],
                                          R[:rows, s0:s0 + L])

                # Row runs within this tile: consecutive rho rows with
                # consecutive target rows share one store DMA.
                for dr, sr, L in _runs(idx[r0:r0 + rows], range(rows)):
                    nc.sync.dma_start(out[dr:dr + L, c0:c1],
                                      W[sr:sr + L, :])
    nc.compile()
    return nc


def kernel(input_state, fock_idx, fock_dim):
    input_state = np.asarray(input_state)
    idx = np.asarray(fock_idx).astype(np.int64)
    D = int(fock_dim)
    B, n, _ = input_state.shape

    nc = _build(idx, D, n)

    out = np.empty((B, D, D), dtype=input_state.dtype)
    for start in range(0, B, N_CORES):
        stop = min(start + N_CORES, B)
        in_maps = [
            {"rho": np.ascontiguousarray(input_state[b], dtype=np.float32)}
            for b in range(start, stop)
        ]
        res = run_bass_kernel_spmd(nc, in_maps,
                                   core_ids=list(range(stop - start)))
        for k, b in enumerate(range(start, stop)):
            out[b] = res.results[k]["out"]
    return out


# revision 4
# speedup vs baseline: 1.0455x; 1.0455x over previous
"""Trainium2 Bass kernel: scatter rho[b, i, j] -> out[b, fock_idx[i], fock_idx[j]].

Sharding: batch dim B across the 8 NeuronCores (pure data parallel). fock_idx is
known on the host at call time, so the scatter addressing is baked into the
compiled program as static DMA/compute access patterns.

Per-core algorithm (out is [D, D], zero except out[idx[i], idx[j]] = rho[i, j]):
  - The runtime hands the NEFF a zero-initialized ExternalOutput buffer (both
    the native run_neff path and the axon/bass2jax donation path guarantee
    this), so only rows/columns that receive data are written.
  - fock_idx (for the real problem) is strictly increasing and decomposes into
    runs of consecutive indices (32 runs of 32). Columns: each rho row is
    expanded into a [span]-wide row in SBUF with the runs placed at their
    target offsets and zeros in the gaps. Rows: each 128-row tile of rho is
    stored with one DMA per row-run to the matching block of out rows,
    touching only columns [c0, c1).
  - The W expansion buffers are memset once up front and reused cyclically:
    the gap columns stay zero across reuse because the per-tile copies only
    ever write the (fixed) data columns.
  - Copies are merged in pairs of runs (per-pair-constant stride 3D APs) and
    spread across Vector/Scalar/GpSimd so the store DMA stream stays the
    critical path.
"""

import numpy as np

import concourse.bacc as bacc
import concourse.bass as bass
import concourse.mybir as mybir
from concourse import tile
from concourse.bass_utils import run_bass_kernel_spmd

N_CORES = 8
P = 128  # SBUF partitions
W_BUFS = 4
R_BUFS = 4


def _runs(dst, src):
    """Maximal runs where dst and src both advance by 1. Yields (d0, s0, len)."""
    out = []
    d0, s0, L = int(dst[0]), int(src[0]), 1
    for k in range(1, len(dst)):
        if int(dst[k]) == d0 + L and int(src[k]) == s0 + L:
            L += 1
        else:
            out.append((d0, s0, L))
            d0, s0, L = int(dst[k]), int(src[k]), 1
    out.append((d0, s0, L))
    return out


def _pair_runs(col_runs):
    """Group adjacent equal-length runs into stride-2 pairs.

    Returns a list of (dst0, src0, pair_dst_stride, pair_src_stride, n, L)
    where n is 1 or 2 repeats of an L-wide copy.
    """
    out = []
    k = 0
    while k < len(col_runs):
        d0, s0, L = col_runs[k]
        if k + 1 < len(col_runs) and col_runs[k + 1][2] == L:
            d1, s1, _ = col_runs[k + 1]
            out.append((d0, s0, d1 - d0, s1 - s0, 2, L))
            k += 2
        else:
            out.append((d0, s0, L, L, 1, L))
            k += 1
    return out


def _build(idx, D, n):
    """Build the per-core Bass program with idx baked in."""
    f32 = mybir.dt.float32

    # Column placement: process columns in sorted-index order so the SBUF row
    # image is written left to right; a run needs source columns contiguous too.
    order = np.argsort(idx, kind="stable")
    col_runs = _runs(idx[order], order)  # (dst_col, src_col, len)
    c0 = min(r[0] for r in col_runs)
    c1 = max(r[0] + r[2] for r in col_runs)
    span = c1 - c0
    pairs = _pair_runs(col_runs)

    nc = bacc.Bacc("TRN2", target_bir_lowering=False, debug=False,
                   num_devices=N_CORES)
    rho = nc.dram_tensor("rho", [n, n], f32, kind="ExternalInput")
    out = nc.dram_tensor("out", [D, D], f32, kind="ExternalOutput")

    n_tiles = (n + P - 1) // P
    with tile.TileContext(nc) as tc:
        with (
            tc.tile_pool(name="rp", bufs=R_BUFS) as rp,
            tc.tile_pool(name="wp", bufs=1) as wp,
            tc.tile_pool(name="scratch", bufs=1) as sp,
        ):
            # DVE ramps up over its first ~5 us of work; burn that on a
            # scratch tile while the first loads are in flight.
            warm = sp.tile([P, 64], f32, name="warm")
            nc.vector.memset(warm[:], 0.0)
            for k in range(19):
                nc.vector.memset(warm[:, (k % 2) * 32:(k % 2) * 32 + 32], 0.0)
            for k in range(4):
                nc.scalar.copy(warm[:, 32:64], warm[:, 0:32])

            ws = []
            for k in range(W_BUFS):
                w = wp.tile([P, span], f32, name=f"W{k}")
                nc.gpsimd.memset(w[:], 0.0)
                ws.append(w)

            for t in range(n_tiles):
                r0 = t * P
                rows = min(P, n - r0)
                R = rp.tile([P, n], f32)
                nc.scalar.dma_start(R[:rows, :], rho[r0:r0 + rows, :])

                W = ws[t % W_BUFS]
                # Spread pair-copies V,V,S,G round-robin (DVE is ~2x the
                # speed of the other two for small f32 copies).
                for i, (d0, s0, ds, ss, cnt, L) in enumerate(pairs):
                    dst = bass.AP(W.tensor, W.offset + (d0 - c0),
                                  [W.ap[0], [ds, cnt], [1, L]])
                    src = bass.AP(R.tensor, R.offset + s0,
                                  [[R.ap[0][0], rows], [ss, cnt], [1, L]])
                    sel = i % 4
                    if sel in (0, 2):
                        nc.vector.tensor_copy(dst, src)
                    elif sel == 1:
                        nc.scalar.copy(dst, src)
                    else:
                        nc.gpsimd.tensor_copy(dst, src)

                # Row runs within this tile: consecutive rho rows with
                # consecutive target rows share one store DMA.
                for dr, sr, L in _runs(idx[r0:r0 + rows], range(rows)):
                    nc.sync.dma_start(out[dr:dr + L, c0:c1],
                                      W[sr:sr + L, :])
    nc.compile()
    return nc


def kernel(input_state, fock_idx, fock_dim):
    input_state = np.asarray(input_state)
    idx = np.asarray(fock_idx).astype(np.int64)
    D = int(fock_dim)
    B, n, _ = input_state.shape

    nc = _build(idx, D, n)

    out = np.empty((B, D, D), dtype=input_state.dtype)
    for start in range(0, B, N_CORES):
        stop = min(start + N_CORES, B)
        in_maps = [
            {"rho": np.ascontiguousarray(input_state[b], dtype=np.float32)}
            for b in range(start, stop)
        ]
        res = run_bass_kernel_spmd(nc, in_maps,
                                   core_ids=list(range(stop - start)))
        for k, b in enumerate(range(start, stop)):
            out[b] = res.results[k]["out"]
    return out


# revision 7
# speedup vs baseline: 1.0655x; 1.0192x over previous
"""Trainium2 Bass kernel: scatter rho[b, i, j] -> out[b, fock_idx[i], fock_idx[j]].

Sharding: batch dim B across the 8 NeuronCores (pure data parallel). fock_idx is
known on the host at call time, so the scatter addressing is baked into the
compiled program as static DMA/compute access patterns.

Per-core algorithm (out is [D, D], zero except out[idx[i], idx[j]] = rho[i, j]):
  - The runtime hands the NEFF a zero-initialized ExternalOutput buffer (both
    the native run_neff path and the axon/bass2jax donation path guarantee
    this), so only rows/columns that receive data are written.
  - fock_idx (for the real problem) is strictly increasing and decomposes into
    runs of consecutive indices (32 runs of 32). Columns: each rho row is
    expanded into a [span]-wide row in SBUF with the runs placed at their
    target offsets and zeros in the gaps. Rows: each 128-row tile of rho is
    stored with one DMA per row-run to the matching block of out rows,
    touching only columns [c0, c1).
  - The W expansion buffers are memset once up front and reused cyclically:
    the gap columns stay zero across reuse because the per-tile copies only
    ever write the (fixed) data columns.
  - Copies are merged in pairs of runs (per-pair-constant stride 3D APs) and
    spread across Vector/Scalar/GpSimd so the store DMA stream stays the
    critical path.
"""

import numpy as np

import concourse.bacc as bacc
import concourse.bass as bass
import concourse.mybir as mybir
from concourse import tile
from concourse.bass_utils import run_bass_kernel_spmd

N_CORES = 8
P = 128  # SBUF partitions
W_BUFS = 4
R_BUFS = 4


def _runs(dst, src):
    """Maximal runs where dst and src both advance by 1. Yields (d0, s0, len)."""
    out = []
    d0, s0, L = int(dst[0]), int(src[0]), 1
    for k in range(1, len(dst)):
        if int(dst[k]) == d0 + L and int(src[k]) == s0 + L:
            L += 1
        else:
            out.append((d0, s0, L))
            d0, s0, L = int(dst[k]), int(src[k]), 1
    out.append((d0, s0, L))
    return out


def _pair_runs(col_runs):
    """Group adjacent equal-length runs into stride-2 pairs.

    Returns a list of (dst0, src0, pair_dst_stride, pair_src_stride, n, L)
    where n is 1 or 2 repeats of an L-wide copy.
    """
    out = []
    k = 0
    while k < len(col_runs):
        d0, s0, L = col_runs[k]
        if k + 1 < len(col_runs) and col_runs[k + 1][2] == L:
            d1, s1, _ = col_runs[k + 1]
            out.append((d0, s0, d1 - d0, s1 - s0, 2, L))
            k += 2
        else:
            out.append((d0, s0, L, L, 1, L))
            k += 1
    return out


def _build(idx, D, n):
    """Build the per-core Bass program with idx baked in."""
    f32 = mybir.dt.float32

    # Column placement: process columns in sorted-index order so the SBUF row
    # image is written left to right; a run needs source columns contiguous too.
    order = np.argsort(idx, kind="stable")
    col_runs = _runs(idx[order], order)  # (dst_col, src_col, len)
    c0 = min(r[0] for r in col_runs)
    c1 = max(r[0] + r[2] for r in col_runs)
    span = c1 - c0
    pairs = _pair_runs(col_runs)

    nc = bacc.Bacc("TRN2", target_bir_lowering=False, debug=False,
                   num_devices=N_CORES)
    rho = nc.dram_tensor("rho", [n, n], f32, kind="ExternalInput")
    out = nc.dram_tensor("out", [D, D], f32, kind="ExternalOutput")

    n_tiles = (n + P - 1) // P
    with tile.TileContext(nc) as tc:
        with (
            tc.tile_pool(name="rp", bufs=R_BUFS) as rp,
            tc.tile_pool(name="wp", bufs=1) as wp,
        ):
            # One-time memsets of the W expansion buffers, split across the
            # vector and gpsimd engines so W0 is ready fast and neither
            # engine's per-tile copy stream starts late.
            ws = []
            for k in range(W_BUFS):
                w = wp.tile([P, span], f32, name=f"W{k}")
                (nc.vector if k % 2 == 0 else nc.gpsimd).memset(w[:], 0.0)
                ws.append(w)

            for t in range(n_tiles):
                r0 = t * P
                rows = min(P, n - r0)
                R = rp.tile([P, n], f32)
                nc.scalar.dma_start(R[:rows, :], rho[r0:r0 + rows, :])

                W = ws[t % W_BUFS]
                # DVE: single-run copies (strided mid-dims halve its rate);
                # Scalar/GpSimd: pair-merged copies (their fixed cost
                # amortizes). Split ~12 runs to V, 10 to S, 10 to G.
                runs_v = [r for k, r in enumerate(col_runs) if k % 8 < 3]
                rest = [r for k, r in enumerate(col_runs) if k % 8 >= 3]
                pr = _pair_runs(rest)
                for d0, s0, L in runs_v:
                    nc.vector.tensor_copy(W[:rows, d0 - c0:d0 - c0 + L],
                                          R[:rows, s0:s0 + L])
                for i, (d0, s0, ds, ss, cnt, L) in enumerate(pr):
                    dst = bass.AP(W.tensor, W.offset + (d0 - c0),
                                  [[W.ap[0][0], rows], [ds, cnt], [1, L]])
                    src = bass.AP(R.tensor, R.offset + s0,
                                  [[R.ap[0][0], rows], [ss, cnt], [1, L]])
                    if i % 2 == 0:
                        nc.scalar.copy(dst, src)
                    else:
                        nc.gpsimd.tensor_copy(dst, src)

                # Row runs within this tile: consecutive rho rows with
                # consecutive target rows share one store DMA.
                for dr, sr, L in _runs(idx[r0:r0 + rows], range(rows)):
                    nc.sync.dma_start(out[dr:dr + L, c0:c1],
                                      W[sr:sr + L, :])
    nc.compile()
    return nc


def kernel(input_state, fock_idx, fock_dim):
    input_state = np.asarray(input_state)
    idx = np.asarray(fock_idx).astype(np.int64)
    D = int(fock_dim)
    B, n, _ = input_state.shape

    nc = _build(idx, D, n)

    out = np.empty((B, D, D), dtype=input_state.dtype)
    for start in range(0, B, N_CORES):
        stop = min(start + N_CORES, B)
        in_maps = [
            {"rho": np.ascontiguousarray(input_state[b], dtype=np.float32)}
            for b in range(start, stop)
        ]
        res = run_bass_kernel_spmd(nc, in_maps,
                                   core_ids=list(range(stop - start)))
        for k, b in enumerate(range(start, stop)):
            out[b] = res.results[k]["out"]
    return out


# revision 8
# speedup vs baseline: 1.0811x; 1.0146x over previous
"""Trainium2 Bass kernel: scatter rho[b, i, j] -> out[b, fock_idx[i], fock_idx[j]].

Sharding: batch dim B across the 8 NeuronCores (pure data parallel). fock_idx is
known on the host at call time, so the scatter addressing is baked into the
compiled program as static DMA/compute access patterns.

Per-core algorithm (out is [D, D], zero except out[idx[i], idx[j]] = rho[i, j]):
  - The runtime hands the NEFF a zero-initialized ExternalOutput buffer (both
    the native run_neff path and the axon/bass2jax donation path guarantee
    this), so only rows/columns that receive data are written.
  - fock_idx (for the real problem) is strictly increasing and decomposes into
    runs of consecutive indices (32 runs of 32). Columns: each rho row is
    expanded into a [span]-wide row in SBUF with the runs placed at their
    target offsets and zeros in the gaps. Rows: each 128-row tile of rho is
    stored with one DMA per row-run to the matching block of out rows,
    touching only columns [c0, c1).
  - The W expansion buffers are memset once up front and reused cyclically:
    the gap columns stay zero across reuse because the per-tile copies only
    ever write the (fixed) data columns.
  - Expansion copies run on Vector (single runs) and GpSimd (pair-merged
    runs); stores alternate between the two HWDGE rings (SP and ACT) so
    DMA issue is not serialized on one sequencer.
"""

import numpy as np

import concourse.bacc as bacc
import concourse.bass as bass
import concourse.mybir as mybir
from concourse import tile
from concourse.bass_utils import run_bass_kernel_spmd

N_CORES = 8
P = 128  # SBUF partitions
W_BUFS = 6
R_BUFS = 4


def _runs(dst, src):
    """Maximal runs where dst and src both advance by 1. Yields (d0, s0, len)."""
    out = []
    d0, s0, L = int(dst[0]), int(src[0]), 1
    for k in range(1, len(dst)):
        if int(dst[k]) == d0 + L and int(src[k]) == s0 + L:
            L += 1
        else:
            out.append((d0, s0, L))
            d0, s0, L = int(dst[k]), int(src[k]), 1
    out.append((d0, s0, L))
    return out


def _pair_runs(col_runs):
    """Group adjacent equal-length runs into stride-2 pairs.

    Returns a list of (dst0, src0, pair_dst_stride, pair_src_stride, n, L)
    where n is 1 or 2 repeats of an L-wide copy.
    """
    out = []
    k = 0
    while k < len(col_runs):
        d0, s0, L = col_runs[k]
        if k + 1 < len(col_runs) and col_runs[k + 1][2] == L:
            d1, s1, _ = col_runs[k + 1]
            out.append((d0, s0, d1 - d0, s1 - s0, 2, L))
            k += 2
        else:
            out.append((d0, s0, L, L, 1, L))
            k += 1
    return out


def _build(idx, D, n):
    """Build the per-core Bass program with idx baked in."""
    f32 = mybir.dt.float32

    # Column placement: process columns in sorted-index order so the SBUF row
    # image is written left to right; a run needs source columns contiguous too.
    order = np.argsort(idx, kind="stable")
    col_runs = _runs(idx[order], order)  # (dst_col, src_col, len)
    c0 = min(r[0] for r in col_runs)
    c1 = max(r[0] + r[2] for r in col_runs)
    span = c1 - c0

    # ~18/32 runs to Vector as singles, rest to GpSimd as pairs.
    runs_v = [r for k, r in enumerate(col_runs) if k % 16 < 9]
    pairs_g = _pair_runs([r for k, r in enumerate(col_runs) if k % 16 >= 9])

    nc = bacc.Bacc("TRN2", target_bir_lowering=False, debug=False,
                   num_devices=N_CORES)
    rho = nc.dram_tensor("rho", [n, n], f32, kind="ExternalInput")
    out = nc.dram_tensor("out", [D, D], f32, kind="ExternalOutput")

    n_tiles = (n + P - 1) // P
    with tile.TileContext(nc) as tc:
        with (
            tc.tile_pool(name="rp", bufs=R_BUFS) as rp,
            tc.tile_pool(name="wp", bufs=1) as wp,
        ):
            # W expansion buffers, memset once and reused cyclically. Emit
            # the first two up front and stagger the rest between the early
            # tile bodies so tile 0's copies are not queued behind them.
            ws = [wp.tile([P, span], f32, name=f"W{k}") for k in range(W_BUFS)]
            memset_order = [(k, nc.vector if k % 2 == 0 else nc.gpsimd)
                            for k in range(W_BUFS)]
            for k, eng in memset_order[:2]:
                eng.memset(ws[k][:], 0.0)
            next_memset = 2

            store_rings = [nc.sync, nc.scalar]
            n_store = 0

            for t in range(n_tiles):
                r0 = t * P
                rows = min(P, n - r0)
                R = rp.tile([P, n], f32)
                nc.sync.dma_start(R[:rows, :], rho[r0:r0 + rows, :])

                W = ws[t % W_BUFS]
                for d0, s0, L in runs_v:
                    nc.vector.tensor_copy(W[:rows, d0 - c0:d0 - c0 + L],
                                          R[:rows, s0:s0 + L])
                for d0, s0, ds, ss, cnt, L in pairs_g:
                    dst = bass.AP(W.tensor, W.offset + (d0 - c0),
                                  [[W.ap[0][0], rows], [ds, cnt], [1, L]])
                    src = bass.AP(R.tensor, R.offset + s0,
                                  [[R.ap[0][0], rows], [ss, cnt], [1, L]])
                    nc.gpsimd.tensor_copy(dst, src)

                # Row runs within this tile: consecutive rho rows with
                # consecutive target rows share one store DMA.
                for dr, sr, L in _runs(idx[r0:r0 + rows], range(rows)):
                    ring = store_rings[n_store % 2]
                    n_store += 1
                    ring.dma_start(out[dr:dr + L, c0:c1], W[sr:sr + L, :],
                                   max_dma_last_dim=1008)

                # Stagger the remaining one-time memsets behind early tiles.
                while next_memset < W_BUFS and next_memset <= t + 2:
                    k, eng = memset_order[next_memset]
                    eng.memset(ws[k][:], 0.0)
                    next_memset += 1
    nc.compile()
    return nc


def kernel(input_state, fock_idx, fock_dim):
    input_state = np.asarray(input_state)
    idx = np.asarray(fock_idx).astype(np.int64)
    D = int(fock_dim)
    B, n, _ = input_state.shape

    nc = _build(idx, D, n)

    out = np.empty((B, D, D), dtype=input_state.dtype)
    for start in range(0, B, N_CORES):
        stop = min(start + N_CORES, B)
        in_maps = [
            {"rho": np.ascontiguousarray(input_state[b], dtype=np.float32)}
            for b in range(start, stop)
        ]
        res = run_bass_kernel_spmd(nc, in_maps,
                                   core_ids=list(range(stop - start)))
        for k, b in enumerate(range(start, stop)):
            out[b] = res.results[k]["out"]
    return out
